# revision 74
# baseline (speedup 1.0000x reference)
"""Trainium2 Bass kernel: PhraseEncodingMixin pairwise span means.

out[b, i, j, h] = (c[b, j, h] - cp[b, i, h]) / (|i - j| + 1)
  c  = cumsum(seq_hiddens, axis=1)
  cp = c shifted right one step (zero padded)

Strategy (8 NeuronCores, i-axis sharded, 32 span-starts per core):
  - r[i, j] = 1 / (|i-j| + 1) precomputed on host (tiny tables).
  - Per (i, j-block): PE computes the rank-1 term r[i, jblk] (x) cp[b, i, :]
    into PSUM with K=32 fp32 selector matmuls (row il of the stationary is
    the r-row, other rows zero; base partition must be 0), batched over all
    4 batch entries (b-major free layout keeps the per-partition scalar
    fixed per op).
  - One fused DVE scalar_tensor_tensor per (i, j-block):
        out = (c * r_col) - psum        # single full-rate pass
  - Output DRAM layout mirrors the SBUF staging layout, so the store DMAs
    are fully contiguous 4 MiB transfers (4 span rows per DMA); the host
    transposes back to [B, L, L, H] when gathering the 8 shards.
"""

import sys

sys.path.insert(0, "/opt/trn_rl_repo")

import numpy as np

B, L, H = 4, 256, 256
NCORES = 8
NI = L // NCORES  # span-start rows per core (32)
P = 128           # SBUF partitions
JB = L // P       # j blocks (2)
BH = B * H        # 1024

IG = 4            # span rows per output DMA group
PADW = 4          # pad columns per staging tile (guard-copy target)
CW = IG * JB * BH + PADW  # staging tile width
GS = (2, 5, 7)    # il % 8 values routed through GPSIMD instead of PE
PE_ILS = [il for il in range(NI) if il % 8 not in GS]
PE_POS = {il: i for i, il in enumerate(PE_ILS)}
GS_ILS = [il for il in range(NI) if il % 8 in GS]
GS_POS = {il: i for i, il in enumerate(GS_ILS)}
# stag groups are PURE (all-PE or all-GPSIMD) so a group's store DMA never
# waits on the other pipeline's tail; out row-blocks follow GROUP order
GROUPS = [PE_ILS[i * IG : (i + 1) * IG] for i in range(len(PE_ILS) // IG)] + [
    GS_ILS[i * IG : (i + 1) * IG] for i in range(len(GS_ILS) // IG)
]
IL_ORDER = [il for grp in GROUPS for il in grp]
# unit emission order: interleave GS units among PE units (ratio 12:20)
_SCHED = []
_acc, _pe_i, _gs_i = 0, 0, 0
for _k in range(NI):
    _acc += len(GS_ILS)
    if _acc >= NI and _gs_i < len(GS_ILS):
        _acc -= NI
        _SCHED.append(GS_ILS[_gs_i])
        _gs_i += 1
    else:
        _SCHED.append(PE_ILS[_pe_i])
        _pe_i += 1
# group id and position within group for each il
_GRP_OF = {}
for _gi, _grp in enumerate(GROUPS):
    for _pos, _il in enumerate(_grp):
        _GRP_OF[_il] = (_gi, _pos)
_cache = {}


def _build_program(walrus_wait_fixups=True, mm_fp32r=False):
    import concourse.bass as bass
    import concourse.mybir as mybir
    from concourse import tile
    from contextlib import ExitStack

    f32 = mybir.dt.float32
    f32r = mybir.dt.float32r
    Alu = mybir.AluOpType

    nc = bass.Bass("TRN2", target_bir_lowering=False, debug=False)

    # cmat = [c | rcol] packed: cmat[p, jb*BH + b*H + h] = c[b, jb*P+p, h],
    # cmat[p, JB*BH + il*JB+jb] = r[i0+il, jb*P+p]. One DMA -> one semaphore
    # source for the DVE ops (walrus limits sync waits per instruction).
    cmat = nc.dram_tensor(
        "cmat", [P, JB * BH + NI * JB], f32, kind="ExternalInput"
    ).ap()
    # wsrc = [cpf | rsel] packed so the PE operands arrive via ONE DMA (the
    # fp32 matmul's LW struct only tolerates a single sync wait). rsel holds
    # selector rows only for the PE-routed span rows.
    WSW = BH + len(PE_ILS) * JB * P
    wsrc = nc.dram_tensor("wsrc", [NI, WSW], f32, kind="ExternalInput").ap()
    # cp rows for the GPSIMD-routed span rows, host-replicated 4x so each
    # lands on quadrant partitions {0,32,64,96} with a plain strided DMA
    cpg = nc.dram_tensor(
        "cpg", [len(GS_ILS) * 4, BH], f32, kind="ExternalInput"
    ).ap()
    # out[ig, p, il'*2048 + b*512 + jb*256 + h] — verbatim staging tiles
    # (each group's 4 pad columns included; host drops them)
    out = nc.dram_tensor(
        "out", [NI // IG, P, CW], f32, kind="ExternalOutput"
    ).ap()

    with tile.TileContext(nc) as tc:
        with ExitStack() as ctx:
            const = ctx.enter_context(tc.tile_pool(name="const", bufs=1))
            psum_pool = ctx.enter_context(
                tc.tile_pool(name="psum", bufs=4, space="PSUM")
            )
            stag_pool = ctx.enter_context(tc.tile_pool(name="stag", bufs=3))
            bc_pool = ctx.enter_context(tc.tile_pool(name="bc", bufs=2))
            diff_pool = ctx.enter_context(tc.tile_pool(name="diff", bufs=3))



            # cumsum tiles, j on partitions, columns [jb][b][h], then rcol
            cmat_sb = const.tile([P, JB * BH + NI * JB], f32)
            nc.sync.dma_start(cmat_sb[:], cmat[:])
            c_sb = cmat_sb[:, : JB * BH]
            rcol_sb = cmat_sb[:, JB * BH :]
            wsrc_sb = const.tile([NI, WSW], f32)
            nc.sync.dma_start(wsrc_sb[:], wsrc[:])
            cpf_sb = wsrc_sb[:, :BH]
            rsel_sb = wsrc_sb[:, BH:]
            # double-buffered quadrant-sparse landing tiles for GS cp rows
            cpq = [const.tile([P, BH], f32, name=f"cpq{i}") for i in range(2)]
            for t in cpq:
                nc.vector.memset(t[:], 0.0)
            # Pool guard: observe the cmat DMA on the Pool engine once, so
            # the tensor_tensor ops below carry only their DVE wait
            pguard = const.tile([P, 1], f32)
            nc.gpsimd.tensor_tensor(
                pguard[:], c_sb[:, 0:1], c_sb[:, 0:1], Alu.add
            )

            D = IG * JB * BH  # first pad column
            open_stag = {}
            for il in _SCHED:
                gi, pos = _GRP_OF[il]
                if gi not in open_stag:
                    stag = stag_pool.tile(
                        [P, CW], f32, name=f"stag{gi}", tag="stag"
                    )
                    open_stag[gi] = [stag, 0]
                    # Guard copy: writes only a pad column (read by the
                    # store DMA, written by no STT). It advances the DVE
                    # vector clock past the slot-release DMA tick and the
                    # const-DMA tick without creating a same-engine WAW
                    # wait, so the ops below carry one sync wait each.
                    nc.vector.tensor_copy(
                        stag[:, D : D + PADW], c_sb[:, 0:PADW]
                    )
                stag = open_stag[gi][0]
                if il % 8 in GS:
                    # GPSIMD path: cp row lands quadrant-sparse via a tiny
                    # DMA, stream_shuffle broadcasts it to all 128
                    # partitions, Pool tensor_tensor computes c - bc, DVE
                    # tensor_scalar applies r.
                    t = GS_POS[il]
                    land = cpq[t % 2]
                    nc.sync.dma_start(
                        bass.AP(
                            land.tensor, land.offset, [[32 * BH, 4], [1, BH]]
                        ),
                        cpg[4 * t : 4 * t + 4, :],
                    )
                    bc = bc_pool.tile([P, BH], f32)
                    nc.vector.stream_shuffle(bc[:], land[:], [0] * 32)
                    for jb in range(JB):
                        g = il * JB + jb
                        diff = diff_pool.tile([P, BH], f32)
                        nc.gpsimd.tensor_tensor(
                            diff[:],
                            c_sb[:, jb * BH : (jb + 1) * BH],
                            bc[:],
                            Alu.subtract,
                        )
                        o0 = (pos * JB + jb) * BH
                        nc.vector.tensor_scalar(
                            stag[:, o0 : o0 + BH],
                            diff[:],
                            rcol_sb[:, g : g + 1],
                            None,
                            Alu.mult,
                        )
                else:
                    for jb in range(JB):
                        psum = psum_pool.tile([P, BH], f32)
                        g = il * JB + jb
                        gp = PE_POS[il] * JB + jb
                        lhsT = rsel_sb[:, gp * P : (gp + 1) * P]
                        for half in range(2):  # N<=512 per PSUM bank
                            rhs = cpf_sb[:, half * 512 : (half + 1) * 512]
                            if mm_fp32r:
                                nc.tensor.matmul(
                                    psum[:, half * 512 : (half + 1) * 512],
                                    lhsT.bitcast(f32r),
                                    rhs.bitcast(f32r),
                                    start=True,
                                    stop=True,
                                )
                            else:
                                nc.tensor.matmul(
                                    psum[:, half * 512 : (half + 1) * 512],
                                    lhsT,
                                    rhs,
                                    start=True,
                                    stop=True,
                                )
                        o0 = (pos * JB + jb) * BH
                        nc.vector.scalar_tensor_tensor(
                            stag[:, o0 : o0 + BH],
                            c_sb[:, jb * BH : (jb + 1) * BH],
                            rcol_sb[:, g : g + 1],
                            psum[:],
                            Alu.mult,
                            Alu.subtract,
                        )
                open_stag[gi][1] += 1
                if open_stag[gi][1] == IG:
                    nc.sync.dma_start(out[gi], stag[:])
                    del open_stag[gi]
            assert not open_stag

    if walrus_wait_fixups:
        # needed for the walrus 1-sync-wait-per-instruction limit; CoreSim's
        # deadlock detector doesn't model the redistributed drain waits
        _strip_redundant_self_waits(nc)
    return nc


_COMPUTE_OPS = {
    "InstMatmult",
    "InstTensorCopy",
    "InstTensorScalarPtr",
    "InstTensorTensor",
    "InstStreamShuffle",
    "InstMemSet",
}
_SELF_PREFIX = {
    "PE": ("PE_",),
    "DVE": ("DVE_",),
    "Activation": ("ACT_", "Activation"),
    "Pool": ("POOL_", "Pool"),
}


def _strip_redundant_self_waits(nc):
    """Reduce every instruction to at most one sync wait (the walrus limit).

    - Compute engines (PE/DVE/ACT/Pool) execute their instruction stream
      strictly in order (the LDWEIGHTS pull-ahead only reorders SBUF reads
      against PSUM writes, which cannot conflict), so waits on the
      instruction's own engine semaphore are implied by program order and
      can be dropped.
    - DMACopy: DMAHW lane waits only order the lane semaphore's previous
      +16; increments commute and the tail drain waits on each lane's
      final value, so they can be dropped when a data-dependency wait
      (compute engine) is present.
    - The kernel-tail drain's lane waits are spread one-per-drain.
    """
    _redistribute_tail_drain_waits(nc)
    for blk in nc.m.functions[0].blocks:
        for inst in blk.instructions:
            nm = type(inst).__name__
            si = inst.sync_info
            if si is None or len(si.on_wait) <= 1:
                continue
            if nm == "InstDMACopy":
                keep = [w for w in si.on_wait if not w.ant_name.startswith("DMAHW")]
                if not keep:
                    keep = list(si.on_wait)[:1]
                assert len(keep) == 1, (nm, si)
                inst.sync_info = type(si)(on_wait=keep, on_update=si.on_update)
                continue
            if nm not in _COMPUTE_OPS:
                continue
            eng = str(inst.engine).split(".")[-1]
            prefixes = _SELF_PREFIX.get(eng)
            assert prefixes is not None, (nm, eng, si)
            keep = [
                w for w in si.on_wait if not w.ant_name.startswith(prefixes)
            ]
            assert len(keep) <= 1, (nm, eng, si)
            inst.sync_info = type(si)(on_wait=keep, on_update=si.on_update)
    # final check: nothing with more than one wait survives
    for blk in nc.m.functions[0].blocks:
        for inst in blk.instructions:
            si = inst.sync_info
            if si is not None and len(si.on_wait) > 1:
                raise AssertionError(
                    f"multi-wait instruction survived: {type(inst).__name__} "
                    f"{inst.name} {si}"
                )


def _redistribute_tail_drain_waits(nc):
    """Spread the kernel-tail drain's DMAHW lane waits one-per-drain.

    Tile's tail drain waits on PE, DVE and all 8 DMAHW lane sems (10 waits;
    walrus allows one sync wait per instruction). The PE/DVE waits are
    causally dominated by the lane waits (every store DMA's sequencer
    waited on the DVE tick, whose last op waited on PE). The 8 lane waits
    are distributed one each onto the tail drain and the per-engine drains
    of the two EVSEM barrier rounds that follow — all of which complete
    before the kernel's final gather — preserving "kernel completes only
    after all stores landed". The drains' existing `>=0` waits are no-op
    placeholders and may be replaced.
    """
    insts = []
    for blk in nc.m.functions[0].blocks:
        insts.extend(blk.instructions)
    tail = None
    for idx, inst in enumerate(insts):
        si = inst.sync_info
        if (
            type(inst).__name__ == "InstDrain"
            and si is not None
            and sum(w.ant_name.startswith("DMAHW") for w in si.on_wait) >= 2
        ):
            tail = idx
            break
    if tail is None:
        return
    d0 = insts[tail]
    si0 = d0.sync_info
    lanes = [w for w in si0.on_wait if w.ant_name.startswith("DMAHW")]
    carriers = [d0]
    for inst in insts[tail + 1 :]:
        if len(carriers) >= len(lanes):
            break
        if type(inst).__name__ != "InstDrain":
            continue
        si = inst.sync_info
        ok = si is None or all(w.wait_value == 0 for w in si.on_wait)
        if ok:
            carriers.append(inst)
    assert len(carriers) >= len(lanes), (len(carriers), len(lanes))
    for inst, lane_wait in zip(carriers, lanes):
        si = inst.sync_info
        upd = list(si.on_update) if si else []
        inst.sync_info = type(si0)(on_wait=[lane_wait], on_update=upd)


import os as _os

MM_FP32R = _os.environ.get("MM_FP32R", "0") == "1"


def _get_nc():
    if "nc" not in _cache:
        _cache["nc"] = _build_program(mm_fp32r=MM_FP32R)
    return _cache["nc"]


def _cumsum_f32(seq):
    """Mirror the reference's fp32 cumsum as closely as possible."""
    try:
        import jax
        import jax.numpy as jnp

        cpu = jax.devices("cpu")[0]
        with jax.default_device(cpu):
            return np.asarray(jnp.cumsum(jnp.asarray(seq), axis=1))
    except Exception:
        return np.cumsum(seq, axis=1, dtype=np.float32)


def _make_in_maps(seq):
    c = _cumsum_f32(seq)
    cp = np.zeros_like(c)
    cp[:, 1:] = c[:, :-1]

    # cmat[p, jb*BH + b*H + h] = c[b, jb*P+p, h] — identical on every core
    cmat = np.ascontiguousarray(
        c.transpose(1, 0, 2)
        .reshape(JB, P, BH)
        .transpose(1, 0, 2)
        .reshape(P, JB * BH)
    ).astype(np.float32)

    jv = np.arange(L)
    in_maps = []
    for k in range(NCORES):
        i0 = k * NI
        iv = i0 + np.arange(NI)
        cpf_k = np.ascontiguousarray(
            cp[:, i0 : i0 + NI].transpose(1, 0, 2).reshape(NI, BH)
        ).astype(np.float32)
        r_k = (
            1.0 / (np.abs(iv[:, None] - jv[None, :]).astype(np.float64) + 1.0)
        ).astype(np.float32)  # [NI, L]
        # wsrc rows = [cpf | rsel]
        WSW = BH + len(PE_ILS) * JB * P
        wsrc_k = np.zeros((NI, WSW), dtype=np.float32)
        wsrc_k[:, :BH] = cpf_k
        # rsel[k, (PE_POS[il]*JB+jb)*P + m] = (k == il) * r_k[il, jb*P+m]
        for il in PE_ILS:
            pp = PE_POS[il]
            wsrc_k[il, BH + pp * L : BH + (pp + 1) * L] = r_k[il]
        # cpg: GPSIMD-routed cp rows, replicated 4x for the quadrant DMA
        cpg_k = np.ascontiguousarray(
            np.repeat(cpf_k[GS_ILS], 4, axis=0)
        ).astype(np.float32)
        # rcol[p, il*JB+jb] = r_k[il, jb*P+p]
        rcol_k = r_k.reshape(NI, JB, P).transpose(2, 0, 1).reshape(P, NI * JB)
        cmat_k = np.ascontiguousarray(np.concatenate([cmat, rcol_k], axis=1))
        in_maps.append({"cmat": cmat_k, "wsrc": wsrc_k, "cpg": cpg_k})
    return in_maps


def _run(seq, trace=False, trace_kwargs=None):
    from concourse.bass_utils import run_bass_kernel_spmd

    nc = _get_nc()
    in_maps = _make_in_maps(seq)
    res = run_bass_kernel_spmd(
        nc,
        in_maps,
        core_ids=list(range(NCORES)),
        trace=trace,
        **(trace_kwargs or {}),
    )
    # per-core out: [NI//IG, P, CW]; cols pos*2048 + jb*1024 + b*256 + h,
    # row blocks in GROUPS order (IL_ORDER maps emitted row -> il)
    allout = np.stack([r["out"] for r in res.results])  # [8, NI//IG, P, CW]
    allout = allout[:, :, :, : IG * JB * BH].reshape(
        NCORES, NI // IG, P, IG, JB, B, H
    )
    arr = allout.transpose(5, 0, 1, 3, 4, 2, 6).reshape(B, NCORES, NI, L, H)
    out = np.empty((B, L, L, H), dtype=np.float32)
    ilo = np.asarray(IL_ORDER)
    for k in range(NCORES):
        out[:, k * NI + ilo] = arr[:, k]
    return out, res


def kernel(**inputs) -> np.ndarray:
    seq = np.asarray(inputs["seq_hiddens"], dtype=np.float32)
    assert seq.shape == (B, L, H), seq.shape
    out, _ = _run(seq)
    return out


# revision 78
# speedup vs baseline: 604.1362x; 604.1362x over previous
"""Trainium2 Bass kernel: PhraseEncodingMixin pairwise span means.

out[b, i, j, h] = (c[b, j, h] - cp[b, i, h]) / (|i - j| + 1)
  c  = cumsum(seq_hiddens, axis=1)
  cp = c shifted right one step (zero padded)

Strategy (8 NeuronCores, i-axis sharded, 32 span-starts per core):
  - r[i, j] = 1 / (|i-j| + 1) precomputed on host (tiny tables).
  - Per (i, j-block): PE computes the rank-1 term r[i, jblk] (x) cp[b, i, :]
    into PSUM with K=32 fp32 selector matmuls (row il of the stationary is
    the r-row, other rows zero; base partition must be 0), batched over all
    4 batch entries (b-major free layout keeps the per-partition scalar
    fixed per op).
  - One fused DVE scalar_tensor_tensor per (i, j-block):
        out = (c * r_col) - psum        # single full-rate pass
  - Output DRAM layout mirrors the SBUF staging layout, so the store DMAs
    are fully contiguous 4 MiB transfers (4 span rows per DMA); the host
    transposes back to [B, L, L, H] when gathering the 8 shards.
"""

import sys

sys.path.insert(0, "/opt/trn_rl_repo")

import numpy as np

B, L, H = 4, 256, 256
NCORES = 8
NI = L // NCORES  # span-start rows per core (32)
P = 128           # SBUF partitions
JB = L // P       # j blocks (2)
BH = B * H        # 1024

IG = 2            # span rows per output DMA group
PADW = 4          # pad columns per staging tile (guard-copy target)
CW = IG * JB * BH + PADW  # staging tile width
GS = (2, 5, 7)    # il % 8 values routed through GPSIMD instead of PE
PE_ILS = [il for il in range(NI) if il % 8 not in GS]
PE_POS = {il: i for i, il in enumerate(PE_ILS)}
GS_ILS = [il for il in range(NI) if il % 8 in GS]
GS_POS = {il: i for i, il in enumerate(GS_ILS)}
# stag groups are PURE (all-PE or all-GPSIMD) so a group's store DMA never
# waits on the other pipeline's tail. Group kinds are interleaved so the
# store stream is paced by the average of both pipelines.
_PEG = [PE_ILS[i * IG : (i + 1) * IG] for i in range(len(PE_ILS) // IG)]
_GSG = [GS_ILS[i * IG : (i + 1) * IG] for i in range(len(GS_ILS) // IG)]
GROUPS = []
_acc, _gs_i, _pe_i = 0, 0, 0
for _k in range(len(_PEG) + len(_GSG)):
    _acc += len(_GSG)
    if _acc >= len(_PEG) + len(_GSG) and _gs_i < len(_GSG):
        _acc -= len(_PEG) + len(_GSG)
        GROUPS.append(_GSG[_gs_i])
        _gs_i += 1
    elif _pe_i < len(_PEG):
        GROUPS.append(_PEG[_pe_i])
        _pe_i += 1
    else:
        GROUPS.append(_GSG[_gs_i])
        _gs_i += 1
IL_ORDER = [il for grp in GROUPS for il in grp]
_SCHED = IL_ORDER  # unit emission follows interleaved group order
# group id and position within group for each il
_GRP_OF = {}
for _gi, _grp in enumerate(GROUPS):
    for _pos, _il in enumerate(_grp):
        _GRP_OF[_il] = (_gi, _pos)
_cache = {}


def _build_program(walrus_wait_fixups=True, mm_fp32r=False):
    import concourse.bass as bass
    import concourse.mybir as mybir
    from concourse import tile
    from contextlib import ExitStack

    f32 = mybir.dt.float32
    f32r = mybir.dt.float32r
    Alu = mybir.AluOpType

    nc = bass.Bass("TRN2", target_bir_lowering=False, debug=False)

    # cmat = [c | rcol] packed: cmat[p, jb*BH + b*H + h] = c[b, jb*P+p, h],
    # cmat[p, JB*BH + il*JB+jb] = r[i0+il, jb*P+p]. One DMA -> one semaphore
    # source for the DVE ops (walrus limits sync waits per instruction).
    cmat = nc.dram_tensor(
        "cmat", [P, JB * BH + NI * JB], f32, kind="ExternalInput"
    ).ap()
    # wsrc = [cpf | rsel] packed so the PE operands arrive via ONE DMA (the
    # fp32 matmul's LW struct only tolerates a single sync wait). rsel holds
    # selector rows only for the PE-routed span rows.
    WSW = BH + len(PE_ILS) * JB * P
    wsrc = nc.dram_tensor("wsrc", [NI, WSW], f32, kind="ExternalInput").ap()
    # cp rows for the GPSIMD-routed span rows, host-replicated 4x so each
    # lands on quadrant partitions {0,32,64,96} with a plain strided DMA
    cpg = nc.dram_tensor(
        "cpg", [len(GS_ILS) * 4, BH], f32, kind="ExternalInput"
    ).ap()
    # out[ig, p, il'*2048 + b*512 + jb*256 + h] — verbatim staging tiles
    # (each group's 4 pad columns included; host drops them)
    out = nc.dram_tensor(
        "out", [NI // IG, P, CW], f32, kind="ExternalOutput"
    ).ap()

    with tile.TileContext(nc) as tc:
        with ExitStack() as ctx:
            const = ctx.enter_context(tc.tile_pool(name="const", bufs=1))
            psum_pool = ctx.enter_context(
                tc.tile_pool(name="psum", bufs=4, space="PSUM")
            )
            stag_pool = ctx.enter_context(tc.tile_pool(name="stag", bufs=5))
            bc_pool = ctx.enter_context(tc.tile_pool(name="bc", bufs=3))
            diff_pool = ctx.enter_context(tc.tile_pool(name="diff", bufs=6))



            # cumsum tiles, j on partitions, columns [jb][b][h], then rcol
            cmat_sb = const.tile([P, JB * BH + NI * JB], f32)
            nc.sync.dma_start(cmat_sb[:], cmat[:])
            c_sb = cmat_sb[:, : JB * BH]
            rcol_sb = cmat_sb[:, JB * BH :]
            wsrc_sb = const.tile([NI, WSW], f32)
            nc.sync.dma_start(wsrc_sb[:], wsrc[:])
            cpf_sb = wsrc_sb[:, :BH]
            rsel_sb = wsrc_sb[:, BH:]
            # double-buffered quadrant-sparse landing tiles for GS cp rows
            cpq = [const.tile([P, BH], f32, name=f"cpq{i}") for i in range(2)]
            for t in cpq:
                nc.vector.memset(t[:], 0.0)
            # Pool guard: observe the cmat DMA on the Pool engine once, so
            # the tensor_tensor ops below carry only their DVE wait
            pguard = const.tile([P, 1], f32)
            nc.gpsimd.tensor_tensor(
                pguard[:], c_sb[:, 0:1], c_sb[:, 0:1], Alu.add
            )

            D = IG * JB * BH  # first pad column
            open_stag = {}
            pending = []  # software pipeline: consumers run one unit late

            def open_group(gi):
                if gi not in open_stag:
                    stag = stag_pool.tile(
                        [P, CW], f32, name=f"stag{gi}", tag="stag"
                    )
                    open_stag[gi] = [stag, 0]
                    # Guard copy: writes only a pad column (read by the
                    # store DMA, written by no STT). It advances the DVE
                    # vector clock past the slot-release DMA tick and the
                    # const-DMA tick without creating a same-engine WAW
                    # wait, so the ops below carry one sync wait each.
                    nc.vector.tensor_copy(
                        stag[:, D : D + PADW], c_sb[:, 0:PADW]
                    )
                return open_stag[gi][0]

            def close_unit(gi):
                open_stag[gi][1] += 1
                if open_stag[gi][1] == IG:
                    nc.sync.dma_start(out[gi], open_stag[gi][0][:])
                    del open_stag[gi]

            def consume_gs(gi, pos, il, diffs):
                stag = open_stag[gi][0]
                for jb in range(JB):
                    g = il * JB + jb
                    o0 = (pos * JB + jb) * BH
                    nc.vector.tensor_scalar(
                        stag[:, o0 : o0 + BH],
                        diffs[jb][:],
                        rcol_sb[:, g : g + 1],
                        None,
                        Alu.mult,
                    )
                close_unit(gi)

            def consume_pe(gi, pos, il, psums):
                stag = open_stag[gi][0]
                for jb in range(JB):
                    g = il * JB + jb
                    o0 = (pos * JB + jb) * BH
                    nc.vector.scalar_tensor_tensor(
                        stag[:, o0 : o0 + BH],
                        c_sb[:, jb * BH : (jb + 1) * BH],
                        rcol_sb[:, g : g + 1],
                        psums[jb][:],
                        Alu.mult,
                        Alu.subtract,
                    )
                close_unit(gi)

            for il in _SCHED:
                gi, pos = _GRP_OF[il]
                open_group(gi)
                if il % 8 in GS:
                    # GPSIMD path: cp row lands quadrant-sparse via a tiny
                    # DMA, stream_shuffle broadcasts it to all 128
                    # partitions, Pool tensor_tensor computes c - bc, DVE
                    # tensor_scalar applies r (one unit later).
                    t = GS_POS[il]
                    land = cpq[t % 2]
                    nc.sync.dma_start(
                        bass.AP(
                            land.tensor, land.offset, [[32 * BH, 4], [1, BH]]
                        ),
                        cpg[4 * t : 4 * t + 4, :],
                    )
                    bc = bc_pool.tile([P, BH], f32)
                    nc.vector.stream_shuffle(bc[:], land[:], [0] * 32)
                    diffs = []
                    for jb in range(JB):
                        diff = diff_pool.tile(
                            [P, BH], f32, name=f"diff_{il}_{jb}", tag="diff"
                        )
                        nc.gpsimd.tensor_tensor(
                            diff[:],
                            c_sb[:, jb * BH : (jb + 1) * BH],
                            bc[:],
                            Alu.subtract,
                        )
                        diffs.append(diff)
                    pending.append((consume_gs, gi, pos, il, diffs))
                else:
                    psums = []
                    for jb in range(JB):
                        psum = psum_pool.tile(
                            [P, BH], f32, name=f"ps_{il}_{jb}", tag="ps"
                        )
                        gp = PE_POS[il] * JB + jb
                        lhsT = rsel_sb[:, gp * P : (gp + 1) * P]
                        for half in range(2):  # N<=512 per PSUM bank
                            rhs = cpf_sb[:, half * 512 : (half + 1) * 512]
                            if mm_fp32r:
                                nc.tensor.matmul(
                                    psum[:, half * 512 : (half + 1) * 512],
                                    lhsT.bitcast(f32r),
                                    rhs.bitcast(f32r),
                                    start=True,
                                    stop=True,
                                )
                            else:
                                nc.tensor.matmul(
                                    psum[:, half * 512 : (half + 1) * 512],
                                    lhsT,
                                    rhs,
                                    start=True,
                                    stop=True,
                                )
                        psums.append(psum)
                    pending.append((consume_pe, gi, pos, il, psums))
                while len(pending) > 1:
                    fn, *args = pending.pop(0)
                    fn(*args)
            while pending:
                fn, *args = pending.pop(0)
                fn(*args)
            assert not open_stag

    if walrus_wait_fixups:
        # needed for the walrus 1-sync-wait-per-instruction limit; CoreSim's
        # deadlock detector doesn't model the redistributed drain waits
        _strip_redundant_self_waits(nc)
    return nc


_COMPUTE_OPS = {
    "InstMatmult",
    "InstTensorCopy",
    "InstTensorScalarPtr",
    "InstTensorTensor",
    "InstStreamShuffle",
    "InstMemSet",
}
_SELF_PREFIX = {
    "PE": ("PE_",),
    "DVE": ("DVE_",),
    "Activation": ("ACT_", "Activation"),
    "Pool": ("POOL_", "Pool"),
}


def _strip_redundant_self_waits(nc):
    """Reduce every instruction to at most one sync wait (the walrus limit).

    - Compute engines (PE/DVE/ACT/Pool) execute their instruction stream
      strictly in order (the LDWEIGHTS pull-ahead only reorders SBUF reads
      against PSUM writes, which cannot conflict), so waits on the
      instruction's own engine semaphore are implied by program order and
      can be dropped.
    - DMACopy: DMAHW lane waits only order the lane semaphore's previous
      +16; increments commute and the tail drain waits on each lane's
      final value, so they can be dropped when a data-dependency wait
      (compute engine) is present.
    - The kernel-tail drain's lane waits are spread one-per-drain.
    """
    _redistribute_tail_drain_waits(nc)
    for blk in nc.m.functions[0].blocks:
        for inst in blk.instructions:
            nm = type(inst).__name__
            si = inst.sync_info
            if si is None or len(si.on_wait) <= 1:
                continue
            if nm == "InstDMACopy":
                keep = [w for w in si.on_wait if not w.ant_name.startswith("DMAHW")]
                if not keep:
                    keep = list(si.on_wait)[:1]
                assert len(keep) == 1, (nm, si)
                inst.sync_info = type(si)(on_wait=keep, on_update=si.on_update)
                continue
            if nm not in _COMPUTE_OPS:
                continue
            eng = str(inst.engine).split(".")[-1]
            prefixes = _SELF_PREFIX.get(eng)
            assert prefixes is not None, (nm, eng, si)
            keep = [
                w for w in si.on_wait if not w.ant_name.startswith(prefixes)
            ]
            assert len(keep) <= 1, (nm, eng, si)
            inst.sync_info = type(si)(on_wait=keep, on_update=si.on_update)
    # final check: nothing with more than one wait survives
    for blk in nc.m.functions[0].blocks:
        for inst in blk.instructions:
            si = inst.sync_info
            if si is not None and len(si.on_wait) > 1:
                raise AssertionError(
                    f"multi-wait instruction survived: {type(inst).__name__} "
                    f"{inst.name} {si}"
                )


def _redistribute_tail_drain_waits(nc):
    """Spread the kernel-tail drain's DMAHW lane waits one-per-drain.

    Tile's tail drain waits on PE, DVE and all 8 DMAHW lane sems (10 waits;
    walrus allows one sync wait per instruction). The PE/DVE waits are
    causally dominated by the lane waits (every store DMA's sequencer
    waited on the DVE tick, whose last op waited on PE). The 8 lane waits
    are distributed one each onto the tail drain and the per-engine drains
    of the two EVSEM barrier rounds that follow — all of which complete
    before the kernel's final gather — preserving "kernel completes only
    after all stores landed". The drains' existing `>=0` waits are no-op
    placeholders and may be replaced.
    """
    insts = []
    for blk in nc.m.functions[0].blocks:
        insts.extend(blk.instructions)
    tail = None
    for idx, inst in enumerate(insts):
        si = inst.sync_info
        if (
            type(inst).__name__ == "InstDrain"
            and si is not None
            and sum(w.ant_name.startswith("DMAHW") for w in si.on_wait) >= 2
        ):
            tail = idx
            break
    if tail is None:
        return
    d0 = insts[tail]
    si0 = d0.sync_info
    lanes = [w for w in si0.on_wait if w.ant_name.startswith("DMAHW")]
    carriers = [d0]
    for inst in insts[tail + 1 :]:
        if len(carriers) >= len(lanes):
            break
        if type(inst).__name__ != "InstDrain":
            continue
        si = inst.sync_info
        ok = si is None or all(w.wait_value == 0 for w in si.on_wait)
        if ok:
            carriers.append(inst)
    assert len(carriers) >= len(lanes), (len(carriers), len(lanes))
    for inst, lane_wait in zip(carriers, lanes):
        si = inst.sync_info
        upd = list(si.on_update) if si else []
        inst.sync_info = type(si0)(on_wait=[lane_wait], on_update=upd)


import os as _os

MM_FP32R = _os.environ.get("MM_FP32R", "0") == "1"


def _get_nc():
    if "nc" not in _cache:
        _cache["nc"] = _build_program(mm_fp32r=MM_FP32R)
    return _cache["nc"]


def _cumsum_f32(seq):
    """Mirror the reference's fp32 cumsum as closely as possible."""
    try:
        import jax
        import jax.numpy as jnp

        cpu = jax.devices("cpu")[0]
        with jax.default_device(cpu):
            return np.asarray(jnp.cumsum(jnp.asarray(seq), axis=1))
    except Exception:
        return np.cumsum(seq, axis=1, dtype=np.float32)


def _make_in_maps(seq):
    c = _cumsum_f32(seq)
    cp = np.zeros_like(c)
    cp[:, 1:] = c[:, :-1]

    # cmat[p, jb*BH + b*H + h] = c[b, jb*P+p, h] — identical on every core
    cmat = np.ascontiguousarray(
        c.transpose(1, 0, 2)
        .reshape(JB, P, BH)
        .transpose(1, 0, 2)
        .reshape(P, JB * BH)
    ).astype(np.float32)

    jv = np.arange(L)
    in_maps = []
    for k in range(NCORES):
        i0 = k * NI
        iv = i0 + np.arange(NI)
        cpf_k = np.ascontiguousarray(
            cp[:, i0 : i0 + NI].transpose(1, 0, 2).reshape(NI, BH)
        ).astype(np.float32)
        r_k = (
            1.0 / (np.abs(iv[:, None] - jv[None, :]).astype(np.float64) + 1.0)
        ).astype(np.float32)  # [NI, L]
        # wsrc rows = [cpf | rsel]
        WSW = BH + len(PE_ILS) * JB * P
        wsrc_k = np.zeros((NI, WSW), dtype=np.float32)
        wsrc_k[:, :BH] = cpf_k
        # rsel[k, (PE_POS[il]*JB+jb)*P + m] = (k == il) * r_k[il, jb*P+m]
        for il in PE_ILS:
            pp = PE_POS[il]
            wsrc_k[il, BH + pp * L : BH + (pp + 1) * L] = r_k[il]
        # cpg: GPSIMD-routed cp rows, replicated 4x for the quadrant DMA
        cpg_k = np.ascontiguousarray(
            np.repeat(cpf_k[GS_ILS], 4, axis=0)
        ).astype(np.float32)
        # rcol[p, il*JB+jb] = r_k[il, jb*P+p]
        rcol_k = r_k.reshape(NI, JB, P).transpose(2, 0, 1).reshape(P, NI * JB)
        cmat_k = np.ascontiguousarray(np.concatenate([cmat, rcol_k], axis=1))
        in_maps.append({"cmat": cmat_k, "wsrc": wsrc_k, "cpg": cpg_k})
    return in_maps


def _run(seq, trace=False, trace_kwargs=None):
    from concourse.bass_utils import run_bass_kernel_spmd

    nc = _get_nc()
    in_maps = _make_in_maps(seq)
    res = run_bass_kernel_spmd(
        nc,
        in_maps,
        core_ids=list(range(NCORES)),
        trace=trace,
        **(trace_kwargs or {}),
    )
    # per-core out: [NI//IG, P, CW]; cols pos*2048 + jb*1024 + b*256 + h,
    # row blocks in GROUPS order (IL_ORDER maps emitted row -> il)
    allout = np.stack([r["out"] for r in res.results])  # [8, NI//IG, P, CW]
    allout = allout[:, :, :, : IG * JB * BH].reshape(
        NCORES, NI // IG, P, IG, JB, B, H
    )
    arr = allout.transpose(5, 0, 1, 3, 4, 2, 6).reshape(B, NCORES, NI, L, H)
    out = np.empty((B, L, L, H), dtype=np.float32)
    ilo = np.asarray(IL_ORDER)
    for k in range(NCORES):
        out[:, k * NI + ilo] = arr[:, k]
    return out, res


def kernel(**inputs) -> np.ndarray:
    seq = np.asarray(inputs["seq_hiddens"], dtype=np.float32)
    assert seq.shape == (B, L, H), seq.shape
    out, _ = _run(seq)
    return out
